# revision 3
# baseline (speedup 1.0000x reference)
"""Cross-correlation layer kernel for Trainium2 (Bass/Tile), SPMD over 8 cores.

Problem: out[b, k, t] = sum_c x1[b, c, t] * x2p[b, c, t + 2D - k]
with x2p = zero-pad(x2, D) along time, D = 10, k in [0, 21).

Full shapes: x1, x2: [16, 512, 8192] fp32 -> out: [16, 21, 8192] fp32.

Sharding: pure data parallel over batch. Each of the 8 cores gets 2 batches
and computes its [2, 21, 8192] slice locally; host concatenates.

Per-core algorithm (all fp32):
  For each time block of 128 (t0), accumulate over 4 channel chunks on the PE:
      G[u, jj] = sum_c x1[c, t0+u] * x2p[c, t0+jj],  u in [0,128), jj in [0,148)
  The needed outputs are the 21 band diagonals  out[20-d, t0+u] = G[u, u+d].
  A per-partition skewed read is not expressible on-chip, so G is dumped to a
  DRAM scratch (row-major [128,148]); there the diagonal becomes a plain
  strided pattern: flat[149*u + d]. One strided DMA gathers [u, (blk, d)]
  tiles back to SBUF (contiguous 84B runs), a PE transpose flips them to
  [(blk, d), u], and one DMA writes 512B-contiguous runs into out[b, k, :].
"""

import numpy as np

import concourse.bass as bass
import concourse.mybir as mybir
import concourse.tile as tile
from concourse import bacc
from concourse.masks import make_identity

D = 10
K = 2 * D + 1  # 21 displacements

F32 = mybir.dt.float32


def build_nc(B, C, T, slab, group, n_cores=8):
    """Build the per-core Bass program for inputs [B, C, T] -> out [B, K, T]."""
    assert C % 128 == 0 and T % slab == 0 and slab % 128 == 0
    nblk_slab = slab // 128
    assert nblk_slab % group == 0
    NCC = C // 128  # channel chunks
    NS = T // slab  # slabs per batch
    NBLK = T // 128  # blocks per batch
    GW = 148  # G width: 128 + 2D
    GF = group * K  # gathered free width per group (<=128 for PE transpose)
    assert GF <= 128

    nc = bacc.Bacc("TRN2", target_bir_lowering=False, num_devices=n_cores)
    x1 = nc.dram_tensor("x1", [B, C, T], F32, kind="ExternalInput")
    x2 = nc.dram_tensor("x2", [B, C, T], F32, kind="ExternalInput")
    out = nc.dram_tensor("out", [B, K, T], F32, kind="ExternalOutput")
    # DRAM scratch holding every block's G tile (row-major [128, GW])
    gdr = nc.dram_tensor("gscratch", [B, NBLK, 128, GW], F32)

    with tile.TileContext(nc) as tc:
        with (
            tc.tile_pool(name="x1p", bufs=2 * NCC) as x1p,
            tc.tile_pool(name="x2p", bufs=2 * NCC) as x2p,
            tc.tile_pool(name="gsb", bufs=4) as gsbp,
            tc.tile_pool(name="diag", bufs=3) as diagp,
            tc.tile_pool(name="outp", bufs=3) as outp,
            tc.tile_pool(name="const", bufs=1) as constp,
            tc.tile_pool(name="ps", bufs=4, space="PSUM") as psp,
            tc.tile_pool(name="pst", bufs=2, space="PSUM") as pstp,
        ):
            ident = constp.tile([128, 128], F32)
            make_identity(nc, ident[:, :])

            for b in range(B):
                for s in range(NS):
                    ts0 = s * slab  # slab start time
                    # ---- load input slabs (bulk traffic, SP/HWDGE ring) ----
                    x1t = [
                        x1p.tile([128, slab], F32, name="x1s", tag="x1s") for _ in range(NCC)
                    ]
                    x2t = [
                        x2p.tile([128, slab + 2 * D], F32, name="x2s", tag="x2s")
                        for _ in range(NCC)
                    ]
                    for cc in range(NCC):
                        c0 = cc * 128
                        nc.sync.dma_start(
                            x1t[cc][:, :], x1[b, c0 : c0 + 128, ts0 : ts0 + slab]
                        )
                        # x2 slab covers x2 time range [ts0 - D, ts0 + slab + D)
                        lo = ts0 - D
                        hi = ts0 + slab + D
                        sb_l = 0
                        if lo < 0:
                            nc.vector.memset(x2t[cc][:, 0 : -lo], 0.0)
                            sb_l = -lo
                            lo = 0
                        sb_r = slab + 2 * D
                        if hi > T:
                            nc.vector.memset(
                                x2t[cc][:, slab + 2 * D - (hi - T) :], 0.0
                            )
                            sb_r = slab + 2 * D - (hi - T)
                            hi = T
                        nc.sync.dma_start(
                            x2t[cc][:, sb_l:sb_r], x2[b, c0 : c0 + 128, lo:hi]
                        )

                    # ---- per 128-block: matmuls -> G psum -> SBUF -> DRAM ----
                    for blk in range(nblk_slab):
                        u0 = blk * 128
                        gps = psp.tile([128, GW], F32, tag="gps")
                        for cc in range(NCC):
                            nc.tensor.matmul(
                                gps[:, :],
                                x1t[cc][:, u0 : u0 + 128],
                                x2t[cc][:, u0 : u0 + GW],
                                start=(cc == 0),
                                stop=(cc == NCC - 1),
                            )
                        gsb = gsbp.tile([128, GW], F32, tag="gsb")
                        nc.vector.tensor_copy(gsb[:, :], gps[:, :])
                        blk_abs = s * nblk_slab + blk
                        nc.scalar.dma_start(gdr[b, blk_abs], gsb[:, :])

                    # ---- per group: diagonal gather -> transpose -> store ----
                    for g in range(nblk_slab // group):
                        blk0 = s * nblk_slab + g * group
                        dtile = diagp.tile([128, GF], F32, tag="diag")
                        # src: flat G rows; element (u, blkd, d) at
                        # base + blkd*128*GW + u*(GW+1) + d
                        src = bass.AP(
                            gdr,
                            (b * NBLK + blk0) * 128 * GW,
                            [[GW + 1, 128], [128 * GW, group], [1, K]],
                        )
                        nc.scalar.dma_start(dtile[:, :], src)
                        # transpose [128, GF] -> [GF, 128]
                        tps = pstp.tile([GF, 128], F32, tag="tps")
                        nc.tensor.transpose(tps[:, :], dtile[:, :], ident[:, :])
                        osb = outp.tile([GF, 128], F32, tag="osb")
                        nc.vector.tensor_copy(osb[:, :], tps[:, :])
                        # out[b, 20-d, blk*128 + u] ; iterate (blkd, d, u)
                        dst = bass.AP(
                            out,
                            (b * K + 2 * D) * T + blk0 * 128,
                            [[128, group], [-T, K], [1, 128]],
                        )
                        nc.scalar.dma_start(dst, osb[:, :])

    nc.compile()
    return nc


_NC_CACHE = {}


def _get_nc(B, C, T, slab, group, n_cores):
    key = (B, C, T, slab, group, n_cores)
    if key not in _NC_CACHE:
        _NC_CACHE[key] = build_nc(B, C, T, slab, group, n_cores=n_cores)
    return _NC_CACHE[key]


def run_sharded(x1, x2, slab=2048, group=4, trace=False, **spmd_kwargs):
    """Run the SPMD kernel on 8 cores over full inputs; returns (out, results)."""
    from concourse.bass_utils import run_bass_kernel_spmd

    n_cores = 8
    Bf, C, T = x1.shape
    assert Bf % n_cores == 0
    Bs = Bf // n_cores
    nc = _get_nc(Bs, C, T, slab, group, n_cores)
    in_maps = [
        {
            "x1": np.ascontiguousarray(x1[i * Bs : (i + 1) * Bs]),
            "x2": np.ascontiguousarray(x2[i * Bs : (i + 1) * Bs]),
        }
        for i in range(n_cores)
    ]
    res = run_bass_kernel_spmd(
        nc, in_maps, core_ids=list(range(n_cores)), trace=trace, **spmd_kwargs
    )
    out = np.concatenate([r["out"] for r in res.results], axis=0)
    return out, res


def kernel(x1, x2):
    x1 = np.asarray(x1, dtype=np.float32)
    x2 = np.asarray(x2, dtype=np.float32)
    out, _ = run_sharded(x1, x2)
    return out


# revision 4
# speedup vs baseline: 1.2352x; 1.2352x over previous
"""Cross-correlation layer kernel for Trainium2 (Bass/Tile), SPMD over 8 cores.

Problem: out[b, k, t] = sum_c x1[b, c, t] * x2p[b, c, t + 2D - k]
with x2p = zero-pad(x2, D) along time, D = 10, k in [0, 21).

Full shapes: x1, x2: [16, 512, 8192] fp32 -> out: [16, 21, 8192] fp32.

Sharding: pure data parallel over batch. Each of the 8 cores gets 2 batches
and computes its [2, 21, 8192] slice locally; host concatenates.

Per-core algorithm:
  Inputs are cast fp32->bf16 during the DMA load (SWDGE cast path); for each
  time block of 128 (t0) the PE accumulates over 4 channel chunks in fp32 PSUM:
      G[u, jj] = sum_c x1[c, t0+u] * x2p[c, t0+jj],  u in [0,128), jj in [0,148)
  The needed outputs are the 21 band diagonals  out[20-d, t0+u] = G[u, u+d].
  A per-partition skewed read is not expressible on-chip, so G blocks are
  staged into a wide SBUF tile (one per slab) and dumped to a DRAM scratch;
  there the diagonal becomes a plain strided pattern. One strided DMA per
  4-block group gathers [u, (blk, d)] tiles back to SBUF (contiguous 84B
  runs), a PE transpose flips them to [(blk, d), u], and one DMA writes
  512B-contiguous runs into out[b, k, :].
"""

import numpy as np

import concourse.bass as bass
import concourse.mybir as mybir
import concourse.tile as tile
from concourse import bacc
from concourse.masks import make_identity

D = 10
K = 2 * D + 1  # 21 displacements

F32 = mybir.dt.float32
BF16 = mybir.dt.bfloat16


def build_nc(B, C, T, slab, group, n_cores=8, in_dt=BF16):
    """Build the per-core Bass program for inputs [B, C, T] -> out [B, K, T]."""
    assert C % 128 == 0 and T % slab == 0 and slab % 128 == 0
    nblk_slab = slab // 128
    assert nblk_slab % group == 0
    NCC = C // 128  # channel chunks
    NS = T // slab  # slabs per batch
    NBLK = T // 128  # blocks per batch
    GW = 148  # G width: 128 + 2D
    SW = nblk_slab * GW  # staged G width per slab
    GF = group * K  # gathered free width per group (<=128 for PE transpose)
    assert GF <= 128

    nc = bacc.Bacc("TRN2", target_bir_lowering=False, num_devices=n_cores)
    x1 = nc.dram_tensor("x1", [B, C, T], F32, kind="ExternalInput")
    x2 = nc.dram_tensor("x2", [B, C, T], F32, kind="ExternalInput")
    out = nc.dram_tensor("out", [B, K, T], F32, kind="ExternalOutput")
    # DRAM scratch: per slab, the 16 G tiles concatenated ([128, 16*148] f32)
    gdr = nc.dram_tensor("gscratch", [B, NS, 128, SW], F32)

    with tile.TileContext(nc) as tc:
        with (
            tc.tile_pool(name="x1p", bufs=2 * NCC) as x1p,
            tc.tile_pool(name="x2p", bufs=2 * NCC) as x2p,
            tc.tile_pool(name="gsb", bufs=2) as gsbp,
            tc.tile_pool(name="diag", bufs=3) as diagp,
            tc.tile_pool(name="outp", bufs=3) as outp,
            tc.tile_pool(name="const", bufs=1) as constp,
            tc.tile_pool(name="ps", bufs=4, space="PSUM") as psp,
            tc.tile_pool(name="pst", bufs=2, space="PSUM") as pstp,
        ):
            ident = constp.tile([128, 128], F32)
            make_identity(nc, ident[:, :])

            for b in range(B):
                for s in range(NS):
                    ts0 = s * slab  # slab start time
                    # ---- load input slabs (SWDGE: casts fp32->bf16 inline) --
                    x1t = [
                        x1p.tile([128, slab], in_dt, name="x1s", tag="x1s")
                        for _ in range(NCC)
                    ]
                    x2t = [
                        x2p.tile([128, slab + 2 * D], in_dt, name="x2s", tag="x2s")
                        for _ in range(NCC)
                    ]
                    for cc in range(NCC):
                        c0 = cc * 128
                        nc.gpsimd.dma_start(
                            x1t[cc][:, :], x1[b, c0 : c0 + 128, ts0 : ts0 + slab]
                        )
                        # x2 slab covers x2 time range [ts0 - D, ts0 + slab + D)
                        lo = ts0 - D
                        hi = ts0 + slab + D
                        sb_l = 0
                        if lo < 0:
                            nc.vector.memset(x2t[cc][:, 0:-lo], 0.0)
                            sb_l = -lo
                            lo = 0
                        sb_r = slab + 2 * D
                        if hi > T:
                            nc.vector.memset(
                                x2t[cc][:, slab + 2 * D - (hi - T) :], 0.0
                            )
                            sb_r = slab + 2 * D - (hi - T)
                            hi = T
                        nc.gpsimd.dma_start(
                            x2t[cc][:, sb_l:sb_r], x2[b, c0 : c0 + 128, lo:hi]
                        )

                    # ---- per 128-block: matmuls -> G psum -> staging tile ----
                    gsb = gsbp.tile([128, SW], F32, name="gsb", tag="gsb")
                    for blk in range(nblk_slab):
                        u0 = blk * 128
                        gps = psp.tile([128, GW], F32, tag="gps")
                        for cc in range(NCC):
                            nc.tensor.matmul(
                                gps[:, :],
                                x1t[cc][:, u0 : u0 + 128],
                                x2t[cc][:, u0 : u0 + GW],
                                start=(cc == 0),
                                stop=(cc == NCC - 1),
                            )
                        nc.vector.tensor_copy(
                            gsb[:, blk * GW : (blk + 1) * GW], gps[:, :]
                        )
                    # one dump per slab (1.2 MB, 9.25KB/partition runs)
                    nc.scalar.dma_start(gdr[b, s], gsb[:, :])

                    # ---- per group: diagonal gather -> transpose -> store ----
                    for g in range(nblk_slab // group):
                        dtile = diagp.tile([128, GF], F32, tag="diag")
                        # src: flat [128, SW] slab dump; element (u, blkd, d) at
                        # base + u*(SW+1) + blkd*GW + d
                        src = bass.AP(
                            gdr,
                            (b * NS + s) * 128 * SW + g * group * GW,
                            [[SW + 1, 128], [GW, group], [1, K]],
                        )
                        nc.scalar.dma_start(dtile[:, :], src)
                        # transpose [128, GF] -> [GF, 128]
                        tps = pstp.tile([GF, 128], F32, tag="tps")
                        nc.tensor.transpose(tps[:, :], dtile[:, :], ident[:, :])
                        osb = outp.tile([GF, 128], F32, tag="osb")
                        nc.vector.tensor_copy(osb[:, :], tps[:, :])
                        # out[b, 20-d, t0 + blkd*128 + u] ; iterate (blkd, d, u)
                        blk0 = s * nblk_slab + g * group
                        dst = bass.AP(
                            out,
                            (b * K + 2 * D) * T + blk0 * 128,
                            [[128, group], [-T, K], [1, 128]],
                        )
                        nc.scalar.dma_start(dst, osb[:, :])

    nc.compile()
    return nc


_NC_CACHE = {}


def _get_nc(B, C, T, slab, group, n_cores, in_dt):
    key = (B, C, T, slab, group, n_cores, in_dt)
    if key not in _NC_CACHE:
        _NC_CACHE[key] = build_nc(B, C, T, slab, group, n_cores=n_cores, in_dt=in_dt)
    return _NC_CACHE[key]


def run_sharded(x1, x2, slab=2048, group=4, in_dt=BF16, trace=False, **spmd_kwargs):
    """Run the SPMD kernel on 8 cores over full inputs; returns (out, results)."""
    from concourse.bass_utils import run_bass_kernel_spmd

    n_cores = 8
    Bf, C, T = x1.shape
    assert Bf % n_cores == 0
    Bs = Bf // n_cores
    nc = _get_nc(Bs, C, T, slab, group, n_cores, in_dt)
    in_maps = [
        {
            "x1": np.ascontiguousarray(x1[i * Bs : (i + 1) * Bs]),
            "x2": np.ascontiguousarray(x2[i * Bs : (i + 1) * Bs]),
        }
        for i in range(n_cores)
    ]
    res = run_bass_kernel_spmd(
        nc, in_maps, core_ids=list(range(n_cores)), trace=trace, **spmd_kwargs
    )
    out = np.concatenate([r["out"] for r in res.results], axis=0)
    return out, res


def kernel(x1, x2):
    x1 = np.asarray(x1, dtype=np.float32)
    x2 = np.asarray(x2, dtype=np.float32)
    out, _ = run_sharded(x1, x2)
    return out


# revision 8
# speedup vs baseline: 1.3057x; 1.0570x over previous
"""Cross-correlation layer kernel for Trainium2 (Bass/Tile), SPMD over 8 cores.

Problem: out[b, k, t] = sum_c x1[b, c, t] * x2p[b, c, t + 2D - k]
with x2p = zero-pad(x2, D) along time, D = 10, k in [0, 21).

Full shapes: x1, x2: [16, 512, 8192] fp32 -> out: [16, 21, 8192] fp32.

Sharding: pure data parallel over batch. Each of the 8 cores gets 2 batches
and computes its [2, 21, 8192] slice locally; host concatenates.

Per-core algorithm:
  Inputs are cast fp32->bf16 during the DMA load (SWDGE cast path); for each
  time block of 128 (t0) the PE accumulates over 4 channel chunks in fp32 PSUM:
      G[u, jj] = sum_c x1[c, t0+u] * x2p[c, t0+jj],  u in [0,128), jj in [0,148)
  The needed outputs are the 21 band diagonals  out[20-d, t0+u] = G[u, u+d].
  A per-partition skewed read is not expressible on-chip, so G blocks are
  staged into a wide SBUF tile (one per slab) and dumped to a DRAM scratch;
  there the diagonal becomes a plain strided pattern. One strided DMA per
  4-block group gathers [u, (blk, d)] tiles back to SBUF (contiguous 84B
  runs), a PE transpose flips them to [(blk, d), u], and one DMA writes
  512B-contiguous runs into out[b, k, :].
"""

import numpy as np

import concourse.bass as bass
import concourse.mybir as mybir
import concourse.tile as tile
from concourse import bacc
from concourse.masks import make_identity

D = 10
K = 2 * D + 1  # 21 displacements

F32 = mybir.dt.float32
F32R = mybir.dt.float32r
BF16 = mybir.dt.bfloat16


def build_nc(B, C, T, slab, group, n_cores=8, mode="bf16"):
    """Build the per-core Bass program for inputs [B, C, T] -> out [B, K, T].

    mode: "bf16" (SWDGE cast loads, bf16 matmul, N=148)
          "f32r" (HWDGE fp32 loads, fp32r matmul, N padded to 256)
    """
    assert C % 128 == 0 and T % slab == 0 and slab % 128 == 0
    nblk_slab = slab // 128
    assert nblk_slab % group == 0
    NCC = C // 128  # channel chunks
    NS = T // slab  # slabs per batch
    NBLK = T // 128  # blocks per batch
    GW = 148  # G width: 128 + 2D
    SW = nblk_slab * GW  # staged G width per slab
    GF = group * K  # gathered free width per group (<=128 for PE transpose)
    assert GF <= 128
    f32r = mode == "f32r"
    in_dt = F32 if f32r else BF16
    # fp32r needs moving dim >= 256 for full rate; extra columns are junk
    MMW = 256 if f32r else GW
    x2w = slab + (128 if f32r else 2 * D)

    nc = bacc.Bacc("TRN2", target_bir_lowering=False, num_devices=n_cores)
    x1 = nc.dram_tensor("x1", [B, C, T], F32, kind="ExternalInput")
    x2 = nc.dram_tensor("x2", [B, C, T], F32, kind="ExternalInput")
    out = nc.dram_tensor("out", [B, K, T], F32, kind="ExternalOutput")
    stg_dt = BF16 if not f32r else F32  # staging/dump/gather dtype
    # DRAM scratch: per slab, the 16 G tiles concatenated ([128, 16*148])
    gdr = nc.dram_tensor("gscratch", [B, NS, 128, SW], stg_dt)

    with tile.TileContext(nc) as tc:
        with (
            tc.tile_pool(name="x1p", bufs=3 * NCC) as x1p,
            tc.tile_pool(name="x2p", bufs=3 * NCC) as x2p,
            tc.tile_pool(name="gsb", bufs=2) as gsbp,
            tc.tile_pool(name="diag", bufs=3) as diagp,
            tc.tile_pool(name="outp", bufs=3) as outp,
            tc.tile_pool(name="const", bufs=1) as constp,
            tc.tile_pool(name="ps", bufs=4, space="PSUM") as psp,
            tc.tile_pool(name="pst", bufs=2, space="PSUM") as pstp,
        ):
            ident = constp.tile([128, 128], stg_dt)
            make_identity(nc, ident[:, :])

            for b in range(B):
                for s in range(NS):
                    ts0 = s * slab  # slab start time
                    # ---- load input slabs (SWDGE: casts fp32->bf16 inline) --
                    x1t = [
                        x1p.tile([128, slab], in_dt, name="x1s", tag="x1s")
                        for _ in range(NCC)
                    ]
                    x2t = [
                        x2p.tile([128, x2w], in_dt, name="x2s", tag="x2s")
                        for _ in range(NCC)
                    ]
                    ldeng = nc.sync if f32r else nc.gpsimd
                    for cc in range(NCC):
                        c0 = cc * 128
                        ldeng.dma_start(
                            x1t[cc][:, :], x1[b, c0 : c0 + 128, ts0 : ts0 + slab]
                        )
                        # x2 tile covers x2 time range [ts0 - D, ts0 - D + x2w),
                        # zero-filled (or junk-pad for f32r tail) out of range
                        lo = ts0 - D
                        lo_c = max(0, lo)
                        hi_c = min(T, lo + x2w)
                        if lo_c > lo:
                            nc.vector.memset(x2t[cc][:, 0 : lo_c - lo], 0.0)
                        if hi_c < lo + x2w:
                            nc.vector.memset(x2t[cc][:, hi_c - lo :], 0.0)
                        ldeng.dma_start(
                            x2t[cc][:, lo_c - lo : hi_c - lo],
                            x2[b, c0 : c0 + 128, lo_c:hi_c],
                        )

                    # ---- per 128-block: matmuls -> G psum -> staging tile ----
                    gsb = gsbp.tile([128, SW], stg_dt, name="gsb", tag="gsb")
                    for blk in range(nblk_slab):
                        u0 = blk * 128
                        gps = psp.tile([128, MMW], F32, tag="gps")
                        for cc in range(NCC):
                            lhs = x1t[cc][:, u0 : u0 + 128]
                            rhs = x2t[cc][:, u0 : u0 + MMW]
                            if f32r:
                                lhs = lhs.bitcast(F32R)
                                rhs = rhs.bitcast(F32R)
                            nc.tensor.matmul(
                                gps[:, :],
                                lhs,
                                rhs,
                                start=(cc == 0),
                                stop=(cc == NCC - 1),
                            )
                        nc.vector.tensor_copy(
                            gsb[:, blk * GW : (blk + 1) * GW], gps[:, 0:GW]
                        )
                    # one dump per slab (1.2 MB, 9.25KB/partition runs)
                    nc.sync.dma_start(gdr[b, s], gsb[:, :])

                    # ---- per group: diagonal gather -> transpose -> store ----
                    for g in range(nblk_slab // group):
                        dtile = diagp.tile([128, GF], stg_dt, tag="diag")
                        # src: flat [128, SW] slab dump; element (u, blkd, d) at
                        # base + u*(SW+1) + blkd*GW + d
                        src = bass.AP(
                            gdr,
                            (b * NS + s) * 128 * SW + g * group * GW,
                            [[SW + 1, 128], [GW, group], [1, K]],
                        )
                        nc.scalar.dma_start(dtile[:, :], src)
                        # transpose [128, GF] -> [GF, 128]
                        tps = pstp.tile([GF, 128], stg_dt, tag="tps")
                        nc.tensor.transpose(tps[:, :], dtile[:, :], ident[:, :])
                        osb = outp.tile([GF, 128], F32, tag="osb")
                        nc.vector.tensor_copy(osb[:, :], tps[:, :])
                        # out[b, 20-d, t0 + blkd*128 + u] ; iterate (blkd, d, u)
                        blk0 = s * nblk_slab + g * group
                        dst = bass.AP(
                            out,
                            (b * K + 2 * D) * T + blk0 * 128,
                            [[128, group], [-T, K], [1, 128]],
                        )
                        nc.sync.dma_start(dst, osb[:, :])

    nc.compile()
    return nc


_NC_CACHE = {}


def _get_nc(B, C, T, slab, group, n_cores, mode):
    key = (B, C, T, slab, group, n_cores, mode)
    if key not in _NC_CACHE:
        _NC_CACHE[key] = build_nc(B, C, T, slab, group, n_cores=n_cores, mode=mode)
    return _NC_CACHE[key]


def run_sharded(x1, x2, slab=2048, group=4, mode="bf16", trace=False, **spmd_kwargs):
    """Run the SPMD kernel on 8 cores over full inputs; returns (out, results)."""
    from concourse.bass_utils import run_bass_kernel_spmd

    n_cores = 8
    Bf, C, T = x1.shape
    assert Bf % n_cores == 0
    Bs = Bf // n_cores
    nc = _get_nc(Bs, C, T, slab, group, n_cores, mode)
    in_maps = [
        {
            "x1": np.ascontiguousarray(x1[i * Bs : (i + 1) * Bs]),
            "x2": np.ascontiguousarray(x2[i * Bs : (i + 1) * Bs]),
        }
        for i in range(n_cores)
    ]
    res = run_bass_kernel_spmd(
        nc, in_maps, core_ids=list(range(n_cores)), trace=trace, **spmd_kwargs
    )
    out = np.concatenate([r["out"] for r in res.results], axis=0)
    return out, res


def kernel(x1, x2):
    x1 = np.asarray(x1, dtype=np.float32)
    x2 = np.asarray(x2, dtype=np.float32)
    out, _ = run_sharded(x1, x2)
    return out


# revision 11
# speedup vs baseline: 1.3676x; 1.0475x over previous
"""Cross-correlation layer kernel for Trainium2 (Bass/Tile), SPMD over 8 cores.

Problem: out[b, k, t] = sum_c x1[b, c, t] * x2p[b, c, t + 2D - k]
with x2p = zero-pad(x2, D) along time, D = 10, k in [0, 21).

Full shapes: x1, x2: [16, 512, 8192] fp32 -> out: [16, 21, 8192] fp32.

Sharding: pure data parallel over batch. Each of the 8 cores gets 2 batches
and computes its [2, 21, 8192] slice locally; host concatenates.

Per-core algorithm:
  Inputs are cast fp32->bf16 during the DMA load (SWDGE cast path); for each
  time block of 128 (t0) the PE accumulates over 4 channel chunks in fp32 PSUM:
      G[u, jj] = sum_c x1[c, t0+u] * x2p[c, t0+jj],  u in [0,128), jj in [0,148)
  The needed outputs are the 21 band diagonals  out[20-d, t0+u] = G[u, u+d].
  A per-partition skewed read is not expressible on-chip, so G blocks are
  staged into a wide SBUF tile (one per slab) and dumped to a DRAM scratch;
  there the diagonal becomes a plain strided pattern. One strided DMA per
  4-block group gathers [u, (blk, d)] tiles back to SBUF (contiguous 84B
  runs), a PE transpose flips them to [(blk, d), u], and one DMA writes
  512B-contiguous runs into out[b, k, :].
"""

import numpy as np

import concourse.bass as bass
import concourse.mybir as mybir
import concourse.tile as tile
from concourse import bacc
from concourse.masks import make_identity

D = 10
K = 2 * D + 1  # 21 displacements

F32 = mybir.dt.float32
F32R = mybir.dt.float32r
BF16 = mybir.dt.bfloat16


def build_nc(B, C, T, slab, group, n_cores=8, mode="bf16", do_mm=True, do_extract=True):
    """Build the per-core Bass program for inputs [B, C, T] -> out [B, K, T].

    mode: "bf16" (SWDGE cast loads, bf16 matmul, N=148)
          "f32r" (HWDGE fp32 loads, fp32r matmul, N padded to 256)
    """
    assert C % 128 == 0 and T % slab == 0 and slab % 128 == 0
    nblk_slab = slab // 128
    assert nblk_slab % group == 0
    NCC = C // 128  # channel chunks
    NS = T // slab  # slabs per batch
    NBLK = T // 128  # blocks per batch
    GW = 148  # G width: 128 + 2D
    SW = nblk_slab * GW  # staged G width per slab
    GF = group * K  # gathered free width per group (<=128 for PE transpose)
    assert GF <= 128
    f32r = mode == "f32r"
    in_dt = F32 if f32r else BF16
    # fp32r needs moving dim >= 256 for full rate; extra columns are junk
    MMW = 256 if f32r else GW
    x2w = slab + (128 if f32r else 2 * D)

    nc = bacc.Bacc("TRN2", target_bir_lowering=False, num_devices=n_cores)
    x1 = nc.dram_tensor("x1", [B, C, T], F32, kind="ExternalInput")
    x2 = nc.dram_tensor("x2", [B, C, T], F32, kind="ExternalInput")
    out = nc.dram_tensor("out", [B, K, T], F32, kind="ExternalOutput")
    stg_dt = BF16 if not f32r else F32  # staging/dump/gather dtype
    # DRAM scratch: per slab, the 16 G tiles concatenated ([128, 16*148])
    gdr = nc.dram_tensor("gscratch", [B, NS, 128, SW], stg_dt)

    with tile.TileContext(nc) as tc:
        with (
            tc.tile_pool(name="x1p", bufs=3 * NCC) as x1p,
            tc.tile_pool(name="x2p", bufs=3 * NCC) as x2p,
            tc.tile_pool(name="gsb", bufs=2) as gsbp,
            tc.tile_pool(name="diag", bufs=3) as diagp,
            tc.tile_pool(name="outp", bufs=3) as outp,
            tc.tile_pool(name="const", bufs=1) as constp,
            tc.tile_pool(name="ps", bufs=4, space="PSUM") as psp,
            tc.tile_pool(name="pst", bufs=2, space="PSUM") as pstp,
        ):
            ident = constp.tile([128, 128], stg_dt)
            make_identity(nc, ident[:, :])

            for b in range(B):
                for s in range(NS):
                    ts0 = s * slab  # slab start time
                    # ---- load input slabs (SWDGE: casts fp32->bf16 inline) --
                    x1t = [
                        x1p.tile([128, slab], in_dt, name="x1s", tag="x1s")
                        for _ in range(NCC)
                    ]
                    x2t = [
                        x2p.tile([128, x2w], in_dt, name="x2s", tag="x2s")
                        for _ in range(NCC)
                    ]
                    ldeng = nc.sync if f32r else nc.gpsimd
                    for cc in range(NCC):
                        c0 = cc * 128
                        ldeng.dma_start(
                            x1t[cc][:, :], x1[b, c0 : c0 + 128, ts0 : ts0 + slab]
                        )
                        # x2 tile covers x2 time range [ts0 - D, ts0 - D + x2w),
                        # zero-filled (or junk-pad for f32r tail) out of range
                        lo = ts0 - D
                        lo_c = max(0, lo)
                        hi_c = min(T, lo + x2w)
                        if lo_c > lo:
                            nc.vector.memset(x2t[cc][:, 0 : lo_c - lo], 0.0)
                        if hi_c < lo + x2w:
                            nc.vector.memset(x2t[cc][:, hi_c - lo :], 0.0)
                        ldeng.dma_start(
                            x2t[cc][:, lo_c - lo : hi_c - lo],
                            x2[b, c0 : c0 + 128, lo_c:hi_c],
                        )

                    # ---- per 128-block: matmuls -> G psum -> staging tile ----
                    gsb = gsbp.tile([128, SW], stg_dt, name="gsb", tag="gsb")
                    for blk in range(nblk_slab if do_mm else 0):
                        u0 = blk * 128
                        gps = psp.tile([128, MMW], F32, tag="gps")
                        for cc in range(NCC):
                            lhs = x1t[cc][:, u0 : u0 + 128]
                            rhs = x2t[cc][:, u0 : u0 + MMW]
                            if f32r:
                                lhs = lhs.bitcast(F32R)
                                rhs = rhs.bitcast(F32R)
                            nc.tensor.matmul(
                                gps[:, :],
                                lhs,
                                rhs,
                                start=(cc == 0),
                                stop=(cc == NCC - 1),
                            )
                        nc.vector.tensor_copy(
                            gsb[:, blk * GW : (blk + 1) * GW], gps[:, 0:GW]
                        )
                    # one dump per slab (1.2 MB, 9.25KB/partition runs)
                    if do_extract:
                        nc.sync.dma_start(gdr[b, s], gsb[:, :])

                    # ---- slab gather: one long run per u covering all 16
                    # blocks' diagonal windows (garbage between windows) ----
                    if do_extract:
                        RW = GW * (nblk_slab - 1) + K  # 2241: run width per u
                        dtile = diagp.tile([128, SW], stg_dt, name="dt", tag="diag")
                        src = bass.AP(
                            gdr,
                            (b * NS + s) * 128 * SW,
                            [[SW + 1, 128], [1, RW]],
                        )
                        nc.scalar.dma_start(dtile[:, 0:RW], src)
                        # dtile[u, GW*bb + d] = G_bb[u, u+d]
                        dview = dtile.rearrange("p (bb j) -> p bb j", j=GW)
                    # ---- per group: pack strided cols, transpose, store ----
                    for g in range(nblk_slab // group if do_extract else 0):
                        # pack [128, (group, K)] strided cols -> contiguous
                        pk = outp.tile([128, GF], stg_dt, name="pk", tag="pk")
                        nc.vector.tensor_copy(
                            pk[:, :], dview[:, g * group : (g + 1) * group, 0:K]
                        )
                        tps = pstp.tile([GF, 128], stg_dt, tag="tps")
                        nc.tensor.transpose(tps[:, :], pk[:, :], ident[:, :])
                        osb = outp.tile([GF, 128], F32, tag="osb")
                        nc.vector.tensor_copy(osb[:, :], tps[:, :])
                        # out[b, 20-d, t0 + blkd*128 + u] ; iterate (blkd, d, u)
                        blk0 = s * nblk_slab + g * group
                        dst = bass.AP(
                            out,
                            (b * K + 2 * D) * T + blk0 * 128,
                            [[128, group], [-T, K], [1, 128]],
                        )
                        nc.sync.dma_start(dst, osb[:, :])

            if not do_extract:
                dummy = constp.tile([128, 16], F32, name="dummy")
                nc.vector.memset(dummy[:, :], 0.0)
                nc.sync.dma_start(
                    bass.AP(out, 0, [[16, 128], [1, 16]]), dummy[:, :]
                )

    nc.compile()
    return nc


_NC_CACHE = {}


def _get_nc(B, C, T, slab, group, n_cores, mode):
    key = (B, C, T, slab, group, n_cores, mode)
    if key not in _NC_CACHE:
        _NC_CACHE[key] = build_nc(B, C, T, slab, group, n_cores=n_cores, mode=mode)
    return _NC_CACHE[key]


def run_sharded(x1, x2, slab=2048, group=4, mode="bf16", trace=False, **spmd_kwargs):
    """Run the SPMD kernel on 8 cores over full inputs; returns (out, results)."""
    from concourse.bass_utils import run_bass_kernel_spmd

    n_cores = 8
    Bf, C, T = x1.shape
    assert Bf % n_cores == 0
    Bs = Bf // n_cores
    nc = _get_nc(Bs, C, T, slab, group, n_cores, mode)
    in_maps = [
        {
            "x1": np.ascontiguousarray(x1[i * Bs : (i + 1) * Bs]),
            "x2": np.ascontiguousarray(x2[i * Bs : (i + 1) * Bs]),
        }
        for i in range(n_cores)
    ]
    res = run_bass_kernel_spmd(
        nc, in_maps, core_ids=list(range(n_cores)), trace=trace, **spmd_kwargs
    )
    out = np.concatenate([r["out"] for r in res.results], axis=0)
    return out, res


def kernel(x1, x2):
    x1 = np.asarray(x1, dtype=np.float32)
    x2 = np.asarray(x2, dtype=np.float32)
    out, _ = run_sharded(x1, x2)
    return out
